# revision 4
# baseline (speedup 1.0000x reference)
"""MinGRU Trainium2 kernel.

Problem: B=8, T=4096, D=512, H=512 MinGRU:
    k = x @ Wz^T + bz;  z = sigmoid(k)
    w = x @ Wh^T + bh;  h~ = g(w),  g(w) = relu(w) + 0.5 (w>=0) | sigmoid(w) (w<0)
    h_t = (1 - z_t) * h_{t-1} + z_t * h~_t,   h_{-1} = g(h_0)
(The reference computes this recurrence in log space via cumlogsumexp; in
linear space all quantities are positive and bounded, so a direct scan with
fp32 state is numerically stable.)

Sharding: data-parallel over batch, one batch row per NeuronCore (8 cores).

Per-core device layout (everything transposed so H sits on partitions and T
on the free dim, which lets the VectorE `tensor_tensor_scan` instruction run
the recurrence along T):
    x8  (D=512, T)  fp8 e4m3 (x * 8)  - z-path GEMM rhs, DoubleRow mode
    xb  (D=512, T)  bf16              - h-path GEMM rhs
    wz8 (D, H) fp8 e4m3 (Wz^T * 32), whb (D, H) bf16 - stationary weights
    kp = x8 @ wz8 accumulated fp8 DoubleRow (2 k-pairs of 256)  [PE]
    wp = xb @ whb bf16                                          [PE]
    a    = sigmoid(-kp/256 - bz)                 [ScalarE, scale+bias fused]
    s    = sigmoid(wp + bh)                      [ScalarE]
    r    = relu(wp + bh)                         [ScalarE]
    g    = min(s, 0.5) + r                       [GpSimd scalar_tensor_tensor]
           (identity: sigmoid(min(v,0)) = min(sigmoid(v), 0.5))
    bn   = (a - 1) * g                           [VectorE scalar_tensor_tensor]
    h    = scan: state = a*state - bn            [VectorE tensor_tensor_scan,
                                                  fp32 internal state]
    hT out (H, T) bf16 -> host transposes back

Mixed precision: the z-path error washes out through the gate (rel err
4.7e-3 in host sim vs 1.4e-2 for both-paths-fp8), so only Wz runs fp8.
The elementwise chain runs in bf16 (DVE 2x packed mode); scan state fp32.
"""

import os

import numpy as np

import concourse.bass as bass
import concourse.mybir as mybir
import concourse.tile as tile
from concourse import bacc
from concourse.bass_utils import run_bass_kernel_spmd

# Problem constants (hardcoded per harness contract).
B, T, D, H = 8, 4096, 512, 512
P = 128          # partitions
DB = D // P      # 4 contraction blocks
HB = H // P      # 4 output h blocks
MM_N = 512       # matmul free-dim chunk (one PSUM bank)
XS = 8.0         # fp8 input scale
WS = 32.0        # fp8 weight scale

F32 = mybir.dt.float32
BF16 = mybir.dt.bfloat16
F8 = mybir.dt.float8e4
EW = BF16        # elementwise chain dtype

# Stash of the last run's BassKernelResults (for test harness introspection).
LAST_RESULT = None


def _build_nc():
    nc = bacc.Bacc(
        "TRN2",
        target_bir_lowering=False,
        debug=False,
        enable_asserts=False,
        num_devices=B,
    )

    x8_d = nc.dram_tensor("x8", (DB, P, T), F8, kind="ExternalInput")
    xb_d = nc.dram_tensor("xb", (DB, P, T), BF16, kind="ExternalInput")
    wz8_d = nc.dram_tensor("wz8", (DB, P, H), F8, kind="ExternalInput")
    whb_d = nc.dram_tensor("whb", (DB, P, H), BF16, kind="ExternalInput")
    # smalls columns: [0:4] -bz per h-block, [4:8] bh, [8:12] g(h_0) carries
    smalls_d = nc.dram_tensor("smalls", (P, 12), F32, kind="ExternalInput")
    hT_d = nc.dram_tensor("hT", (H, T), EW, kind="ExternalOutput")

    AF = mybir.ActivationFunctionType
    OP = mybir.AluOpType
    DR = mybir.MatmulPerfMode.DoubleRow

    from contextlib import ExitStack

    with tile.TileContext(nc) as tc, ExitStack() as ctx:
        wpool = ctx.enter_context(tc.tile_pool(name="weights", bufs=1))
        xpool = ctx.enter_context(tc.tile_pool(name="xres", bufs=1))
        spool = ctx.enter_context(tc.tile_pool(name="work", bufs=4))
        ppool = ctx.enter_context(tc.tile_pool(name="psum", bufs=2, space="PSUM"))

        # --- Resident tensors: whole x (fp8 + bf16) and both weight sets.
        smalls = wpool.tile([P, 12], F32, name="smalls")
        nc.gpsimd.dma_start(smalls[:], smalls_d.ap()[:])

        wz8_sb = wpool.tile([P, DB, H], F8, name="wz8_sb")
        whb_sb = wpool.tile([P, DB, H], BF16, name="whb_sb")
        x8_sb = xpool.tile([P, DB, T], F8, name="x8_sb")
        xb_sb = xpool.tile([P, DB, T], BF16, name="xb_sb")

        C0 = 512  # first chunk: small to prime the pipeline fast
        # First-chunk x slabs + weights first (the first GEMMs need them),
        # spread across the sync/scalar HWDGE rings so descriptor
        # generation (~0.6us/DMA within one ring) overlaps.
        for db in range(DB):
            nc.sync.dma_start(x8_sb[:, db, :C0], x8_d.ap()[db][:, :C0])
        for db in range(DB):
            nc.scalar.dma_start(wz8_sb[:, db, :], wz8_d.ap()[db])
        for db in range(DB):
            nc.sync.dma_start(xb_sb[:, db, :C0], xb_d.ap()[db][:, :C0])
        for db in range(DB):
            nc.scalar.dma_start(whb_sb[:, db, :], whb_d.ap()[db])
        # Rest of x: big slabs, one DMA each (rows spread across HW queues);
        # the chunk-1 region (C0:C0+1024) first so the tc=1 GEMMs don't wait
        # on the whole-tensor transfer.
        for db in range(DB):
            nc.sync.dma_start(x8_sb[:, db, C0:], x8_d.ap()[db][:, C0:])
        for db in range(DB):
            nc.scalar.dma_start(
                xb_sb[:, db, C0:C0 + 1024], xb_d.ap()[db][:, C0:C0 + 1024]
            )
        for db in range(DB):
            nc.scalar.dma_start(
                xb_sb[:, db, C0 + 1024:], xb_d.ap()[db][:, C0 + 1024:]
            )

        # PE p-state warmup: stream dummy matmuls on zeroed tiles while the
        # setup DMAs are in flight so the clock is at 2.4GHz when real
        # matmuls start (cold PE runs at ~1/3 speed for the first ~3us).
        dwa = wpool.tile([P, 128], BF16, name="dwa")
        nc.gpsimd.memset(dwa[:], 0.0)
        dwb = wpool.tile([P, 512], BF16, name="dwb")
        nc.gpsimd.memset(dwb[:], 0.0)
        for _ in range(20):
            dp = ppool.tile([P, 512], F32, name="dp", tag="kp")
            nc.tensor.matmul(dp[:], dwa[:], dwb[:], start=True, stop=True)

        # --- Main loops: h-block outer (weights stay hot), T chunks inner
        # (small first chunk primes the pipeline; small last chunk shortens
        # the serial tail) ---
        CHUNKS = [C0, 1024, 1024, 1024, 512]
        assert sum(CHUNKS) == T
        starts = [sum(CHUNKS[:i]) for i in range(len(CHUNKS))]

        for hb in range(HB):
            hs = slice(hb * P, (hb + 1) * P)
            for ci, (ts0, clen) in enumerate(zip(starts, CHUNKS)):
                # z-path GEMM: fp8 DoubleRow, 2 k-pairs of 256 contraction
                kp = ppool.tile([P, 1024], F32, name="kp", tag="kp")
                for cc in range(0, clen, MM_N):
                    cs = slice(ts0 + cc, ts0 + cc + MM_N)
                    for p2 in (0, 2):
                        nc.tensor.matmul(
                            kp[:, cc:cc + MM_N],
                            wz8_sb[:, p2:p2 + 2, hs],
                            x8_sb[:, p2:p2 + 2, cs],
                            start=(p2 == 0), stop=(p2 == 2),
                            perf_mode=DR,
                        )

                a_t = spool.tile([P, 1024], EW, name="a_t", tag="a")
                nc.scalar.activation(
                    a_t[:, :clen], kp[:, :clen], AF.Sigmoid,
                    bias=smalls[:, hb:hb + 1], scale=-1.0 / (XS * WS),
                )

                # h-path GEMM: bf16
                wp = ppool.tile([P, 1024], F32, name="wp", tag="wp")
                for db in range(DB):
                    for cc in range(0, clen, MM_N):
                        cs = slice(ts0 + cc, ts0 + cc + MM_N)
                        nc.tensor.matmul(
                            wp[:, cc:cc + MM_N],
                            whb_sb[:, db, hs],
                            xb_sb[:, db, cs],
                            start=(db == 0), stop=(db == DB - 1),
                        )

                s_t = spool.tile([P, 1024], EW, name="s_t", tag="s")
                r_t = spool.tile([P, 1024], EW, name="r_t", tag="r")
                nc.scalar.activation(
                    s_t[:, :clen], wp[:, :clen], AF.Sigmoid,
                    bias=smalls[:, 4 + hb:5 + hb], scale=1.0,
                )
                nc.scalar.activation(
                    r_t[:, :clen], wp[:, :clen], AF.Relu,
                    bias=smalls[:, 4 + hb:5 + hb], scale=1.0,
                )

                # m = min(s, 0.5) on the (otherwise idle) GpSimd engine;
                # Pool's ISA has tensor_scalar but not min-tensor_tensor/stt.
                m_t = spool.tile([P, 1024], EW, name="m_t", tag="m")
                nc.gpsimd.tensor_scalar(
                    m_t[:, :clen], s_t[:, :clen], 0.5, None, op0=OP.min,
                )
                g_t = spool.tile([P, 1024], EW, name="g_t", tag="g")
                nc.vector.tensor_tensor(
                    g_t[:, :clen], m_t[:, :clen], r_t[:, :clen], op=OP.add,
                )
                # bn = (a - 1) * g; scan computes a*state - bn
                #    = a*state + (1-a)*g
                bn_t = spool.tile([P, 1024], EW, name="bn_t", tag="bn")
                nc.vector.scalar_tensor_tensor(
                    bn_t[:, :clen], a_t[:, :clen], 1.0, g_t[:, :clen],
                    op0=OP.subtract, op1=OP.mult,
                )

                h_t = spool.tile([P, 1024], EW, name="h_t", tag="h")
                last_tile = (ci == len(CHUNKS) - 1) and (hb == HB - 1)
                nscan = 4 if last_tile else 1
                ssub = clen // nscan
                for u in range(nscan):
                    us = slice(u * ssub, (u + 1) * ssub)
                    init = (smalls[:, 8 + hb:9 + hb] if u == 0
                            else h_t[:, u * ssub - 1:u * ssub])
                    nc.vector.tensor_tensor_scan(
                        h_t[:, us], a_t[:, us], bn_t[:, us], init,
                        op0=OP.mult, op1=OP.subtract,
                    )
                    nc.sync.dma_start(
                        hT_d.ap()[hs, ts0 + u * ssub:ts0 + (u + 1) * ssub],
                        h_t[:, us],
                    )
                if ci + 1 < len(CHUNKS):
                    nc.gpsimd.tensor_copy(
                        smalls[:, 8 + hb:9 + hb], h_t[:, clen - 1:clen]
                    )

    nc.compile()
    return nc


def _host_prep(x, h_0, Wz, bz, Wh, bh):
    x = np.asarray(x, dtype=np.float32)
    h_0 = np.asarray(h_0, dtype=np.float32)
    Wz = np.asarray(Wz, dtype=np.float32)
    bz = np.asarray(bz, dtype=np.float32)
    Wh = np.asarray(Wh, dtype=np.float32)
    bh = np.asarray(bh, dtype=np.float32)

    import ml_dtypes
    bf16 = ml_dtypes.bfloat16
    f8 = ml_dtypes.float8_e4m3
    xT = np.transpose(x, (0, 2, 1))                      # (B, D, T)
    xb = np.ascontiguousarray(xT.astype(bf16)).reshape(B, DB, P, T)
    x8 = np.ascontiguousarray((xT * XS).astype(f8)).reshape(B, DB, P, T)
    wz8 = np.ascontiguousarray((Wz.T * WS).reshape(DB, P, H).astype(f8))
    whb = np.ascontiguousarray(Wh.T.reshape(DB, P, H).astype(bf16))

    # initial carry: g(h_0) = min(sigmoid(h_0), 0.5) + relu(h_0)
    sig = 1.0 / (1.0 + np.exp(-h_0.astype(np.float64)))
    h0g = (np.minimum(sig, 0.5) + np.maximum(h_0, 0.0)).astype(np.float32)

    smalls = np.zeros((B, P, 12), dtype=np.float32)
    for hb in range(HB):
        blk = slice(hb * P, (hb + 1) * P)
        smalls[:, :, hb] = -bz[blk]
        smalls[:, :, 4 + hb] = bh[blk]
        smalls[:, :, 8 + hb] = h0g[:, blk]
    smalls = np.ascontiguousarray(smalls)

    in_maps = []
    for i in range(B):
        in_maps.append({
            "x8": x8[i],
            "xb": xb[i],
            "wz8": wz8,
            "whb": whb,
            "smalls": smalls[i],
        })
    return in_maps


def kernel(x, h_0, Wz, bz, Wh, bh):
    global LAST_RESULT
    in_maps = _host_prep(x, h_0, Wz, bz, Wh, bh)
    nc = _build_nc()
    res = run_bass_kernel_spmd(
        nc,
        in_maps,
        core_ids=list(range(B)),
        trace=bool(int(os.environ.get("MINGRU_TRACE", "0"))),
    )
    LAST_RESULT = res
    out = np.empty((B, T, H), dtype=np.float32)
    for i in range(B):
        out[i] = np.asarray(res.results[i]["hT"]).astype(np.float32).T
    return out


# revision 6
# speedup vs baseline: 2.9869x; 2.9869x over previous
"""MinGRU Trainium2 kernel.

Problem: B=8, T=4096, D=512, H=512 MinGRU:
    k = x @ Wz^T + bz;  z = sigmoid(k)
    w = x @ Wh^T + bh;  h~ = g(w),  g(w) = relu(w) + 0.5 (w>=0) | sigmoid(w) (w<0)
    h_t = (1 - z_t) * h_{t-1} + z_t * h~_t,   h_{-1} = g(h_0)
(The reference computes this recurrence in log space via cumlogsumexp; in
linear space all quantities are positive and bounded, so a direct scan with
fp32 state is numerically stable.)

Sharding: data-parallel over batch, one batch row per NeuronCore (8 cores).

Per-core device layout (everything transposed so H sits on partitions and T
on the free dim, which lets the VectorE `tensor_tensor_scan` instruction run
the recurrence along T):
    x8  (D=512, T)  fp8 e4m3 (x * 8)  - z-path GEMM rhs, DoubleRow mode
    xb  (D=512, T)  bf16              - h-path GEMM rhs
    wz8 (D, H) fp8 e4m3 (Wz^T * 32), whb (D, H) bf16 - stationary weights
    kp = x8 @ wz8 accumulated fp8 DoubleRow (2 k-pairs of 256)  [PE]
    wp = xb @ whb bf16                                          [PE]
    a    = sigmoid(-kp/256 - bz)                 [ScalarE, scale+bias fused]
    s    = sigmoid(wp + bh)                      [ScalarE]
    r    = relu(wp + bh)                         [ScalarE]
    g    = min(s, 0.5) + r                       [GpSimd scalar_tensor_tensor]
           (identity: sigmoid(min(v,0)) = min(sigmoid(v), 0.5))
    bn   = (a - 1) * g                           [VectorE scalar_tensor_tensor]
    h    = scan: state = a*state - bn            [VectorE tensor_tensor_scan,
                                                  fp32 internal state]
    hT out (H, T) bf16 -> host transposes back

Mixed precision: the z-path error washes out through the gate (rel err
4.7e-3 in host sim vs 1.4e-2 for both-paths-fp8), so only Wz runs fp8.
The elementwise chain runs in bf16 (DVE 2x packed mode); scan state fp32.
"""

import os

import numpy as np

import concourse.bass as bass
import concourse.mybir as mybir
import concourse.tile as tile
from concourse import bacc
from concourse.bass_utils import run_bass_kernel_spmd

# Problem constants (hardcoded per harness contract).
B, T, D, H = 8, 4096, 512, 512
P = 128          # partitions
DB = D // P      # 4 contraction blocks
HB = H // P      # 4 output h blocks
MM_N = 512       # matmul free-dim chunk (one PSUM bank)
XS = 8.0         # fp8 input scale
WS = 32.0        # fp8 weight scale

F32 = mybir.dt.float32
BF16 = mybir.dt.bfloat16
F8 = mybir.dt.float8e4
EW = BF16        # elementwise chain dtype

# Stash of the last run's BassKernelResults (for test harness introspection).
LAST_RESULT = None


def _build_nc():
    nc = bacc.Bacc(
        "TRN2",
        target_bir_lowering=False,
        debug=False,
        enable_asserts=False,
        num_devices=B,
    )

    x8_d = nc.dram_tensor("x8", (DB, P, T), F8, kind="ExternalInput")
    xb_d = nc.dram_tensor("xb", (DB, P, T), BF16, kind="ExternalInput")
    wz8_d = nc.dram_tensor("wz8", (DB, P, H), F8, kind="ExternalInput")
    whb_d = nc.dram_tensor("whb", (DB, P, H), BF16, kind="ExternalInput")
    # smalls columns: [0:4] -bz per h-block, [4:8] bh, [8:12] g(h_0)
    # carries, [12:16] +bz
    smalls_d = nc.dram_tensor("smalls", (P, 16), F32, kind="ExternalInput")
    hT_d = nc.dram_tensor("hT", (H, T), EW, kind="ExternalOutput")

    AF = mybir.ActivationFunctionType
    OP = mybir.AluOpType
    DR = mybir.MatmulPerfMode.DoubleRow

    from contextlib import ExitStack

    with tile.TileContext(nc) as tc, ExitStack() as ctx:
        wpool = ctx.enter_context(tc.tile_pool(name="weights", bufs=1))
        xpool = ctx.enter_context(tc.tile_pool(name="xres", bufs=1))
        spool = ctx.enter_context(tc.tile_pool(name="work", bufs=4))
        ppool = ctx.enter_context(tc.tile_pool(name="psum", bufs=2, space="PSUM"))

        # --- Resident tensors: whole x (fp8 + bf16) and both weight sets.
        smalls = wpool.tile([P, 16], F32, name="smalls")
        nc.gpsimd.dma_start(smalls[:], smalls_d.ap()[:])

        wz8_sb = wpool.tile([P, DB, H], F8, name="wz8_sb")
        whb_sb = wpool.tile([P, DB, H], BF16, name="whb_sb")
        x8_sb = xpool.tile([P, DB, T], F8, name="x8_sb")
        xb_sb = xpool.tile([P, DB, T], BF16, name="xb_sb")

        C0 = 512  # first chunk: small to prime the pipeline fast
        # First-chunk x slabs + weights first (the first GEMMs need them),
        # spread across the sync/scalar HWDGE rings so descriptor
        # generation (~0.6us/DMA within one ring) overlaps.
        for db in range(DB):
            nc.sync.dma_start(x8_sb[:, db, :C0], x8_d.ap()[db][:, :C0])
        for db in range(DB):
            nc.scalar.dma_start(wz8_sb[:, db, :], wz8_d.ap()[db])
        for db in range(DB):
            nc.sync.dma_start(xb_sb[:, db, :C0], xb_d.ap()[db][:, :C0])
        for db in range(DB):
            nc.scalar.dma_start(whb_sb[:, db, :], whb_d.ap()[db])
        # Rest of x: big slabs, one DMA each (rows spread across HW queues);
        # the chunk-1 region (C0:C0+1024) first so the tc=1 GEMMs don't wait
        # on the whole-tensor transfer.
        for db in range(DB):
            nc.sync.dma_start(x8_sb[:, db, C0:], x8_d.ap()[db][:, C0:])
        for db in range(DB):
            nc.scalar.dma_start(
                xb_sb[:, db, C0:C0 + 1024], xb_d.ap()[db][:, C0:C0 + 1024]
            )
        for db in range(DB):
            nc.scalar.dma_start(
                xb_sb[:, db, C0 + 1024:], xb_d.ap()[db][:, C0 + 1024:]
            )

        # PE p-state warmup: stream dummy matmuls on zeroed tiles while the
        # setup DMAs are in flight so the clock is at 2.4GHz when real
        # matmuls start (cold PE runs at ~1/3 speed for the first ~3us).
        dwa = wpool.tile([P, 128], BF16, name="dwa")
        nc.gpsimd.memset(dwa[:], 0.0)
        dwb = wpool.tile([P, 512], BF16, name="dwb")
        nc.gpsimd.memset(dwb[:], 0.0)
        for _ in range(20):
            dp = ppool.tile([P, 512], F32, name="dp", tag="kp")
            nc.tensor.matmul(dp[:], dwa[:], dwb[:], start=True, stop=True)

        # --- Main loops: T chunks outer (the 4 h-blocks' scan chains stay
        # independent, so consecutive DVE scans never wait on each other),
        # h-block inner. Small first chunk primes the pipeline; small last
        # chunk shortens the serial tail. ---
        CHUNKS = [C0, 1024, 1024, 1024, 512]
        assert sum(CHUNKS) == T
        starts = [sum(CHUNKS[:i]) for i in range(len(CHUNKS))]

        for ci, (ts0, clen) in enumerate(zip(starts, CHUNKS)):
            for hb in range(HB):
                hs = slice(hb * P, (hb + 1) * P)
                # Scalar/DVE balance: Scalar does the 4th pass (z) on most
                # tiles; on ci==2 DVE derives t=a-1 instead (tensor_scalar
                # runs 4x-packed, so it is cheap there).
                use_z = ci != 2

                # z-path GEMM: fp8 DoubleRow, 2 k-pairs of 256 contraction
                kp = ppool.tile([P, 1024], F32, name="kp", tag="kp")
                for cc in range(0, clen, MM_N):
                    cs = slice(ts0 + cc, ts0 + cc + MM_N)
                    for p2 in (0, 2):
                        nc.tensor.matmul(
                            kp[:, cc:cc + MM_N],
                            wz8_sb[:, p2:p2 + 2, hs],
                            x8_sb[:, p2:p2 + 2, cs],
                            start=(p2 == 0), stop=(p2 == 2),
                            perf_mode=DR,
                        )

                a_t = spool.tile([P, 1024], EW, name="a_t", tag="a")
                nc.scalar.activation(
                    a_t[:, :clen], kp[:, :clen], AF.Sigmoid,
                    bias=smalls[:, hb:hb + 1], scale=-1.0 / (XS * WS),
                )
                if use_z:
                    z_t = spool.tile([P, 1024], EW, name="z_t", tag="z")
                    nc.scalar.activation(
                        z_t[:, :clen], kp[:, :clen], AF.Sigmoid,
                        bias=smalls[:, 12 + hb:13 + hb], scale=1.0 / (XS * WS),
                    )

                # h-path GEMM: bf16
                wp = ppool.tile([P, 1024], F32, name="wp", tag="wp")
                for db in range(DB):
                    for cc in range(0, clen, MM_N):
                        cs = slice(ts0 + cc, ts0 + cc + MM_N)
                        nc.tensor.matmul(
                            wp[:, cc:cc + MM_N],
                            whb_sb[:, db, hs],
                            xb_sb[:, db, cs],
                            start=(db == 0), stop=(db == DB - 1),
                        )

                s_t = spool.tile([P, 1024], EW, name="s_t", tag="s")
                r_t = spool.tile([P, 1024], EW, name="r_t", tag="r")
                nc.scalar.activation(
                    s_t[:, :clen], wp[:, :clen], AF.Sigmoid,
                    bias=smalls[:, 4 + hb:5 + hb], scale=1.0,
                )
                nc.scalar.activation(
                    r_t[:, :clen], wp[:, :clen], AF.Relu,
                    bias=smalls[:, 4 + hb:5 + hb], scale=1.0,
                )

                # g = min(s, 0.5) + r; then bn with either z (Scalar-made)
                # or t=a-1 (DVE-made). All DVE ops below run packed
                # (tensor_scalar 4x / tensor_tensor 2x at bf16).
                m_t = spool.tile([P, 1024], EW, name="m_t", tag="m")
                nc.vector.tensor_scalar_min(m_t[:, :clen], s_t[:, :clen], 0.5)
                g_t = spool.tile([P, 1024], EW, name="g_t", tag="g")
                nc.vector.tensor_add(g_t[:, :clen], m_t[:, :clen], r_t[:, :clen])
                bn_t = spool.tile([P, 1024], EW, name="bn_t", tag="bn")
                if use_z:
                    nc.vector.tensor_mul(bn_t[:, :clen], z_t[:, :clen], g_t[:, :clen])
                else:
                    t_t = spool.tile([P, 1024], EW, name="t_t", tag="t")
                    nc.vector.tensor_scalar_sub(t_t[:, :clen], a_t[:, :clen], 1.0)
                    nc.vector.tensor_mul(bn_t[:, :clen], t_t[:, :clen], g_t[:, :clen])

                h_t = spool.tile([P, 1024], EW, name="h_t", tag="h")
                op1 = OP.add if use_z else OP.subtract
                last_tile = (ci == len(CHUNKS) - 1) and (hb == HB - 1)
                nscan = 4 if last_tile else 1
                ssub = clen // nscan
                for u in range(nscan):
                    us = slice(u * ssub, (u + 1) * ssub)
                    init = (smalls[:, 8 + hb:9 + hb] if u == 0
                            else h_t[:, u * ssub - 1:u * ssub])
                    nc.vector.tensor_tensor_scan(
                        h_t[:, us], a_t[:, us], bn_t[:, us], init,
                        op0=OP.mult, op1=op1,
                    )
                    nc.sync.dma_start(
                        hT_d.ap()[hs, ts0 + u * ssub:ts0 + (u + 1) * ssub],
                        h_t[:, us],
                    )
                if ci + 1 < len(CHUNKS):
                    nc.vector.tensor_copy(
                        smalls[:, 8 + hb:9 + hb], h_t[:, clen - 1:clen]
                    )

    nc.compile()
    return nc


def _host_prep(x, h_0, Wz, bz, Wh, bh):
    x = np.asarray(x, dtype=np.float32)
    h_0 = np.asarray(h_0, dtype=np.float32)
    Wz = np.asarray(Wz, dtype=np.float32)
    bz = np.asarray(bz, dtype=np.float32)
    Wh = np.asarray(Wh, dtype=np.float32)
    bh = np.asarray(bh, dtype=np.float32)

    import ml_dtypes
    bf16 = ml_dtypes.bfloat16
    f8 = ml_dtypes.float8_e4m3
    xT = np.transpose(x, (0, 2, 1))                      # (B, D, T)
    xb = np.ascontiguousarray(xT.astype(bf16)).reshape(B, DB, P, T)
    x8 = np.ascontiguousarray((xT * XS).astype(f8)).reshape(B, DB, P, T)
    wz8 = np.ascontiguousarray((Wz.T * WS).reshape(DB, P, H).astype(f8))
    whb = np.ascontiguousarray(Wh.T.reshape(DB, P, H).astype(bf16))

    # initial carry: g(h_0) = min(sigmoid(h_0), 0.5) + relu(h_0)
    sig = 1.0 / (1.0 + np.exp(-h_0.astype(np.float64)))
    h0g = (np.minimum(sig, 0.5) + np.maximum(h_0, 0.0)).astype(np.float32)

    smalls = np.zeros((B, P, 16), dtype=np.float32)
    for hb in range(HB):
        blk = slice(hb * P, (hb + 1) * P)
        smalls[:, :, hb] = -bz[blk]
        smalls[:, :, 4 + hb] = bh[blk]
        smalls[:, :, 8 + hb] = h0g[:, blk]
        smalls[:, :, 12 + hb] = bz[blk]
    smalls = np.ascontiguousarray(smalls)

    in_maps = []
    for i in range(B):
        in_maps.append({
            "x8": x8[i],
            "xb": xb[i],
            "wz8": wz8,
            "whb": whb,
            "smalls": smalls[i],
        })
    return in_maps


def kernel(x, h_0, Wz, bz, Wh, bh):
    global LAST_RESULT
    in_maps = _host_prep(x, h_0, Wz, bz, Wh, bh)
    nc = _build_nc()
    res = run_bass_kernel_spmd(
        nc,
        in_maps,
        core_ids=list(range(B)),
        trace=bool(int(os.environ.get("MINGRU_TRACE", "0"))),
    )
    LAST_RESULT = res
    out = np.empty((B, T, H), dtype=np.float32)
    for i in range(B):
        out[i] = np.asarray(res.results[i]["hT"]).astype(np.float32).T
    return out
